# revision 13
# baseline (speedup 1.0000x reference)
"""Bass/Trainium2 kernel for the CME linear-recurrence module.

Math per (t, b, f, s, v):
    x = -alpha[t,b,f] * s[s,v];  hh = exp(x);  ih = exprel(x)
    b_t = fs * ih;  F_t = hh_t F_{t-1} + b_t  (scan over t)
    til[t,b,f,s] = Re(sum_v eta_v F_t) * (tau_s/N)^G / tau_s

Device factorization: A_t = cumsum(alpha), theta_t = s_im A_t,
d_t = exp(-alpha s_re) (real).  F_t = e^{-i theta_t} K G_t with K = -1/s and
    G_t = d_t G_{t-1} + g_t,
    g_t = (fs/alpha) (d_t e^{i theta_{t-1}} - e^{i theta_t})        [exact path]
    g_t = fs (-s)(1 + x/2 + x^2/6) e^{i theta_t}     for |x| < 0.05 [series path]
K is t-independent and the scan is linear, so K is folded into the host-side
output tables (eta' K) instead of multiplying g.  The real-coefficient
recurrence runs as two hardware scans per v (tensor_tensor_scan).
cos/sin(theta) use ACT Sin after Cody-Waite reduction (Sin domain [-pi, pi]);
d uses a degree-6 polynomial on a v-independent [128, T] tile, so the ACT
engine only ever touches the Sin table (no act-table switches).

Sharding: data-parallel over flattened (b, f): 256 pairs -> 32 per core.
Per-core layout: partition p = s_lo*32 + n_local (s_lo in 0..3), loop s_hi in
0..7 (s = s_hi*4 + s_lo), free dims (v=12, t=256).  Big [128, 12, T]
elementwise passes are split between DVE and Pool; the 24 per-v scans run
12/12 on DVE/Pool.  Walrus caveat: STT-family instructions (tensor_scalar /
scalar_tensor_tensor / scans) can encode only a couple of sync waits, so
multi-DMA-produced tiles are consolidated through a tensor_copy first.
"""

import numpy as np

import concourse.bass as bass
import concourse.bacc as bacc
import concourse.tile as tile
from concourse import mybir
from concourse.bass_utils import run_bass_kernel_spmd

T, B, F, S, V = 256, 4, 64, 32, 12
NCORES = 8
BF = B * F
NL = BF // NCORES     # 32
SL = 4
SH = S // SL          # 8
NCONST = 11.0
GEXP = 1.0

MAGIC = 12582912.0
INV2PI = float(np.float32(1.0 / (2.0 * np.pi)))
C1 = 6.28125
C2 = float(np.float32(2.0 * np.pi - 6.28125))
PI_CLAMP = 3.1415925
ALPHA_FLOOR = 1e-10
SMALL_X = 0.05

# exp(x) on [-1, 0], degree-6 power-basis coeffs (rel err ~ 4e-8)
_cheb = np.polynomial.chebyshev.Chebyshev.interpolate(np.exp, 6, domain=[-1.0, 0.0])
EXP_COEF = [float(c) for c in _cheb.convert(kind=np.polynomial.Polynomial).coef]

fp32 = mybir.dt.float32
u8 = mybir.dt.uint8
Alu = mybir.AluOpType
Act = mybir.ActivationFunctionType


def _bcast_v(t, nv=V, ncols=T):
    """[128, ncols] tile -> AP [128, nv, ncols] broadcast over v (stride 0)."""
    ap = [list(x) for x in t.ap]
    return bass.AP(tensor=t.tensor, offset=t.offset, ap=[ap[0], [0, nv], ap[1]])


def _bcast_t(t, sh, ncols):
    """[128, SH, V] table tile -> AP [128, V, ncols] for a given s_hi."""
    ap = [list(x) for x in t.ap]
    v_step = ap[2][0]
    return bass.AP(tensor=t.tensor, offset=t.offset + sh * ap[1][0],
                   ap=[ap[0], [v_step, V], [0, ncols]])


def _swap_vt(t):
    """[128, V, T] tile -> AP [128, T, V] (v innermost) for reduce over v."""
    ap = [list(x) for x in t.ap]
    return bass.AP(tensor=t.tensor, offset=t.offset, ap=[ap[0], ap[2], ap[1]])


TAB_NAMES = ["sim", "thr", "er", "ei", "p0r", "p0i", "p1r", "p1i", "p2r", "p2i",
             "kre", "kim"]


def _build(nc):
    # inputs pre-transposed AND pre-replicated x4 host-side: [128, T], so each
    # is a single contiguous DMA (walrus sync-wait limit forbids fan-in)
    al_d = nc.dram_tensor("alpha_sh", [128, T], fp32, kind="ExternalInput").ap()
    fs_d = nc.dram_tensor("fs_sh", [128, T], fp32, kind="ExternalInput").ap()
    tabs_d = {n: nc.dram_tensor(f"tab_{n}", [SH, 128, V], fp32, kind="ExternalInput").ap()
              for n in TAB_NAMES}
    srn_d = nc.dram_tensor("tab_sren1", [SH, 128], fp32, kind="ExternalInput").ap()
    til_d = nc.dram_tensor("til_o", [S, NL, T], fp32, kind="ExternalOutput").ap()
    fl_d = nc.dram_tensor("fl_o", [2, S, NL, V], fp32, kind="ExternalOutput").ap()

    with tile.TileContext(nc) as tc:
        with (
            tc.tile_pool(name="singles", bufs=1) as singles,
            tc.tile_pool(name="angles", bufs=1) as angles,
            tc.tile_pool(name="work", bufs=1) as work,
            tc.tile_pool(name="outp", bufs=2) as outp,
        ):
            alb = singles.tile([128, T], fp32, tag="alb")
            nc.gpsimd.dma_start(out=alb, in_=bass.AP(
                tensor=al_d.tensor, offset=0, ap=[[T, 128], [1, T]]))
            fsb = singles.tile([128, T], fp32, tag="fsb")
            nc.gpsimd.dma_start(out=fsb, in_=bass.AP(
                tensor=fs_d.tensor, offset=0, ap=[[T, 128], [1, T]]))

            tabs = {}
            for n in TAB_NAMES:
                tt = singles.tile([128, SH, V], fp32, tag=f"tab_{n}")
                src = bass.AP(tensor=tabs_d[n].tensor, offset=0,
                              ap=[[V, 128], [128 * V, SH], [1, V]])
                nc.gpsimd.dma_start(out=tt, in_=src)
                tabs[n] = tt
            sren1 = singles.tile([128, SH], fp32, tag="sren1")
            nc.gpsimd.dma_start(out=sren1, in_=bass.AP(
                tensor=srn_d.tensor, offset=0, ap=[[1, 128], [128, SH]]))

            ones = singles.tile([128, T], fp32, tag="ones")
            nc.vector.memset(ones, 1.0)
            halfpi = singles.tile([128, 1], fp32, tag="halfpi")
            nc.vector.memset(halfpi, float(np.pi / 2))

            tilacc = singles.tile([128, SH, T], fp32, tag="tilacc")
            flacc_re = singles.tile([128, SH, V], fp32, tag="flacc_re")
            flacc_im = singles.tile([128, SH, V], fp32, tag="flacc_im")
            A_ext = singles.tile([128, T + 1], fp32, tag="A_ext")
            nc.vector.memset(A_ext[:, 0:1], 0.0)
            nc.vector.tensor_tensor_scan(out=A_ext[:, 1:], data0=ones, data1=alb,
                                         initial=0.0, op0=Alu.mult, op1=Alu.add)

            alc = singles.tile([128, T], fp32, tag="alc")
            nc.vector.tensor_scalar_max(out=alc, in0=alb, scalar1=ALPHA_FLOOR)
            ra = singles.tile([128, T], fp32, tag="ra")
            nc.vector.reciprocal(out=ra, in_=alc)
            wq = singles.tile([128, T], fp32, tag="wq")
            nc.vector.tensor_tensor(out=wq, in0=fsb, in1=ra, op=Alu.mult)

            TE = T + 1
            for sh in range(SH):
                sim_b = _bcast_t(tabs["sim"], sh, TE)
                th = angles.tile([128, V, TE], fp32, tag="th")
                nc.vector.tensor_tensor(out=th, in0=sim_b, in1=_bcast_v(A_ext, V, TE),
                                        op=Alu.mult)
                r = angles.tile([128, V, TE], fp32, tag="r")
                nc.vector.tensor_scalar_mul(out=r, in0=th, scalar1=INV2PI)
                k1 = angles.tile([128, V, TE], fp32, tag="k1")
                nc.vector.tensor_scalar(out=k1, in0=r, scalar1=MAGIC, scalar2=MAGIC,
                                        op0=Alu.add, op1=Alu.subtract)
                k2 = angles.tile([128, V, TE], fp32, tag="k2")
                nc.vector.tensor_scalar(out=k2, in0=r, scalar1=0.25, scalar2=MAGIC,
                                        op0=Alu.add, op1=Alu.add)
                nc.vector.tensor_scalar_sub(out=k2, in0=k2, scalar1=MAGIC)
                nc.vector.scalar_tensor_tensor(out=r, in0=k1, scalar=-C1, in1=th,
                                               op0=Alu.mult, op1=Alu.add)
                nc.vector.scalar_tensor_tensor(out=r, in0=k1, scalar=-C2, in1=r,
                                               op0=Alu.mult, op1=Alu.add)
                nc.vector.tensor_scalar(out=r, in0=r, scalar1=PI_CLAMP, scalar2=-PI_CLAMP,
                                        op0=Alu.min, op1=Alu.max)
                nc.vector.scalar_tensor_tensor(out=k1, in0=k2, scalar=-C1, in1=th,
                                               op0=Alu.mult, op1=Alu.add)
                nc.vector.scalar_tensor_tensor(out=k1, in0=k2, scalar=-C2, in1=k1,
                                               op0=Alu.mult, op1=Alu.add)
                nc.vector.tensor_scalar(out=k1, in0=k1, scalar1=PI_CLAMP / 2,
                                        scalar2=-3 * 3.1415927 / 2,
                                        op0=Alu.min, op1=Alu.max)
                cn_s = angles.tile([128, V, TE], fp32, tag="th")
                nc.scalar.activation(out=cn_s, in_=r, func=Act.Sin)
                cn_c = angles.tile([128, V, TE], fp32, tag="k2")
                nc.scalar.activation(out=cn_c, in_=k1, func=Act.Sin, bias=halfpi)

                c_prev, c_cur = cn_c[:, :, 0:T], cn_c[:, :, 1:TE]
                n_prev, n_cur = cn_s[:, :, 0:T], cn_s[:, :, 1:TE]

                # d = exp(-alpha*s_re): v-independent small polynomial
                xre = work.tile([128, T], fp32, tag="xre")
                nc.vector.tensor_scalar_mul(out=xre, in0=alb,
                                            scalar1=sren1[:, sh:sh + 1])
                d = work.tile([128, T], fp32, tag="d")
                nc.vector.tensor_scalar(out=d, in0=xre, scalar1=EXP_COEF[6],
                                        scalar2=EXP_COEF[5], op0=Alu.mult, op1=Alu.add)
                for ci in (4, 3, 2, 1, 0):
                    nc.vector.tensor_tensor(out=d, in0=d, in1=xre, op=Alu.mult)
                    nc.vector.tensor_scalar_add(out=d, in0=d, scalar1=EXP_COEF[ci])
                D2 = work.tile([128, T], fp32, tag="D2")
                nc.vector.tensor_tensor(out=D2, in0=wq, in1=d, op=Alu.mult)

                # g' = w*(d e^{i th_prev} - e^{i th_cur})   [scan source, exact]
                g_re = work.tile([128, V, T], fp32, tag="g_re")
                nc.vector.tensor_tensor(out=g_re, in0=_bcast_v(D2), in1=c_prev, op=Alu.mult)
                t_re = work.tile([128, V, T], fp32, tag="t_re")
                nc.vector.tensor_tensor(out=t_re, in0=_bcast_v(wq), in1=c_cur, op=Alu.mult)
                nc.vector.tensor_tensor(out=g_re, in0=g_re, in1=t_re, op=Alu.subtract)
                g_im = work.tile([128, V, T], fp32, tag="g_im")
                nc.vector.tensor_tensor(out=g_im, in0=_bcast_v(D2), in1=n_prev, op=Alu.mult)
                nc.vector.tensor_tensor(out=t_re, in0=_bcast_v(wq), in1=n_cur, op=Alu.mult)
                nc.vector.tensor_tensor(out=g_im, in0=g_im, in1=t_re, op=Alu.subtract)

                # series branch (|x| < SMALL_X): g' = fs*(P0 + a*(P1 + a*P2))*e^{i th}
                mask = work.tile([128, V, T], u8, tag="mask")
                nc.vector.tensor_tensor(out=mask, in0=_bcast_v(alb),
                                        in1=_bcast_t(tabs["thr"], sh, T), op=Alu.is_lt)
                qr = work.tile([128, V, T], fp32, tag="qr")
                nc.vector.tensor_tensor(out=qr, in0=_bcast_v(alb),
                                        in1=_bcast_t(tabs["p2r"], sh, T), op=Alu.mult)
                nc.vector.tensor_tensor(out=qr, in0=qr,
                                        in1=_bcast_t(tabs["p1r"], sh, T), op=Alu.add)
                nc.vector.tensor_tensor(out=qr, in0=qr, in1=_bcast_v(alb), op=Alu.mult)
                nc.vector.tensor_tensor(out=qr, in0=qr,
                                        in1=_bcast_t(tabs["p0r"], sh, T), op=Alu.add)
                qi = work.tile([128, V, T], fp32, tag="qi")
                nc.gpsimd.tensor_tensor(out=qi, in0=_bcast_v(alb),
                                        in1=_bcast_t(tabs["p2i"], sh, T), op=Alu.mult)
                nc.gpsimd.tensor_tensor(out=qi, in0=qi,
                                        in1=_bcast_t(tabs["p1i"], sh, T), op=Alu.add)
                nc.gpsimd.tensor_tensor(out=qi, in0=qi, in1=_bcast_v(alb), op=Alu.mult)
                nc.gpsimd.tensor_tensor(out=qi, in0=qi,
                                        in1=_bcast_t(tabs["p0i"], sh, T), op=Alu.add)
                gs_re = work.tile([128, V, T], fp32, tag="gs_re")
                nc.vector.tensor_tensor(out=gs_re, in0=qr, in1=c_cur, op=Alu.mult)
                t_im = work.tile([128, V, T], fp32, tag="t_im")
                nc.gpsimd.tensor_tensor(out=t_im, in0=qi, in1=n_cur, op=Alu.mult)
                nc.vector.tensor_tensor(out=gs_re, in0=gs_re, in1=t_im, op=Alu.subtract)
                nc.vector.tensor_tensor(out=gs_re, in0=gs_re, in1=_bcast_v(fsb), op=Alu.mult)
                gs_im = work.tile([128, V, T], fp32, tag="gs_im")
                nc.gpsimd.tensor_tensor(out=gs_im, in0=qr, in1=n_cur, op=Alu.mult)
                nc.gpsimd.tensor_tensor(out=t_im, in0=qi, in1=c_cur, op=Alu.mult)
                nc.gpsimd.tensor_tensor(out=gs_im, in0=gs_im, in1=t_im, op=Alu.add)
                nc.gpsimd.tensor_tensor(out=gs_im, in0=gs_im, in1=_bcast_v(fsb), op=Alu.mult)
                nc.vector.copy_predicated(out=g_re, mask=mask, data=gs_re)
                nc.vector.copy_predicated(out=g_im, mask=mask, data=gs_im)

                # scans over t: G'_re on DVE, G'_im on Pool
                G_re = work.tile([128, V, T], fp32, tag="G_re")
                G_im = work.tile([128, V, T], fp32, tag="G_im")
                for v in range(V):
                    nc.vector.tensor_tensor_scan(out=G_re[:, v, :], data0=d,
                                                 data1=g_re[:, v, :], initial=0.0,
                                                 op0=Alu.mult, op1=Alu.add)
                    nc.vector.tensor_tensor_scan(out=G_im[:, v, :], data0=d,
                                                 data1=g_im[:, v, :], initial=0.0,
                                                 op0=Alu.mult, op1=Alu.add)

                # F_last = e^{-i theta_T} K G'_T (tiny, into accumulators)
                fre = outp.tile([128, V], fp32, tag="fre")
                fim = outp.tile([128, V], fp32, tag="fim")
                ftmp = outp.tile([128, V], fp32, tag="ftmp")
                ftmp2 = outp.tile([128, V], fp32, tag="ftmp2")
                kre_b = _bcast_t(tabs["kre"], sh, 1)
                kim_b = _bcast_t(tabs["kim"], sh, 1)
                kre2 = bass.AP(tensor=kre_b.tensor, offset=kre_b.offset,
                               ap=[kre_b.ap[0], kre_b.ap[1]])
                kim2 = bass.AP(tensor=kim_b.tensor, offset=kim_b.offset,
                               ap=[kim_b.ap[0], kim_b.ap[1]])
                cl, nl_ = cn_c[:, :, TE - 1], cn_s[:, :, TE - 1]
                Grl, Gil = G_re[:, :, T - 1], G_im[:, :, T - 1]
                nc.vector.tensor_tensor(out=fre, in0=Grl, in1=kre2, op=Alu.mult)
                nc.vector.tensor_tensor(out=ftmp, in0=Gil, in1=kim2, op=Alu.mult)
                nc.vector.tensor_tensor(out=fre, in0=fre, in1=ftmp, op=Alu.subtract)
                nc.vector.tensor_tensor(out=fim, in0=Grl, in1=kim2, op=Alu.mult)
                nc.vector.tensor_tensor(out=ftmp, in0=Gil, in1=kre2, op=Alu.mult)
                nc.vector.tensor_tensor(out=fim, in0=fim, in1=ftmp, op=Alu.add)
                nc.vector.tensor_tensor(out=ftmp, in0=cl, in1=fre, op=Alu.mult)
                nc.vector.tensor_tensor(out=ftmp2, in0=nl_, in1=fim, op=Alu.mult)
                nc.vector.tensor_tensor(out=flacc_re[:, sh, :], in0=ftmp, in1=ftmp2, op=Alu.add)
                nc.vector.tensor_tensor(out=ftmp2, in0=cl, in1=fim, op=Alu.mult)
                nc.vector.tensor_tensor(out=fim, in0=nl_, in1=fre, op=Alu.mult)
                nc.vector.tensor_tensor(out=flacc_im[:, sh, :], in0=ftmp2, in1=fim, op=Alu.subtract)

                # til = sum_v c*(er G_re - ei G_im) + n*(ei G_re + er G_im)
                # er/ei tables already include K and the tau scale (host).
                # ops reading G_re stay on DVE, ops reading G_im on Pool, so
                # each instruction waits on few cross-engine producers.
                er_b = _bcast_t(tabs["er"], sh, T)
                ei_b = _bcast_t(tabs["ei"], sh, T)
                uu = work.tile([128, V, T], fp32, tag="t_re")
                nc.vector.tensor_tensor(out=uu, in0=G_re, in1=er_b, op=Alu.mult)
                tmp = work.tile([128, V, T], fp32, tag="qr")
                nc.gpsimd.tensor_tensor(out=tmp, in0=G_im, in1=ei_b, op=Alu.mult)
                nc.vector.tensor_tensor(out=uu, in0=uu, in1=tmp, op=Alu.subtract)
                ww = work.tile([128, V, T], fp32, tag="qi")
                nc.gpsimd.tensor_tensor(out=ww, in0=G_im, in1=er_b, op=Alu.mult)
                tmp2 = work.tile([128, V, T], fp32, tag="gs_re")
                nc.vector.tensor_tensor(out=tmp2, in0=G_re, in1=ei_b, op=Alu.mult)
                nc.gpsimd.tensor_tensor(out=ww, in0=ww, in1=tmp2, op=Alu.add)
                nc.vector.tensor_tensor(out=uu, in0=uu, in1=c_cur, op=Alu.mult)
                nc.gpsimd.tensor_tensor(out=ww, in0=ww, in1=n_cur, op=Alu.mult)
                nc.vector.tensor_tensor(out=uu, in0=uu, in1=ww, op=Alu.add)
                nc.vector.tensor_reduce(out=tilacc[:, sh, :], in_=_swap_vt(uu),
                                        axis=mybir.AxisListType.X, op=Alu.add)

            # batched output DMAs: 4 for til, 8 for F_last
            for sl in range(SL):
                rows = slice(32 * sl, 32 * (sl + 1))
                # til_o[s, n, t] with s = sh*4 + sl
                dst = bass.AP(tensor=til_d.tensor, offset=sl * NL * T,
                              ap=[[T, NL], [SL * NL * T, SH], [1, T]])
                nc.sync.dma_start(out=dst, in_=tilacc[rows, :, :])
                dst_re = bass.AP(tensor=fl_d.tensor, offset=sl * NL * V,
                                 ap=[[V, NL], [SL * NL * V, SH], [1, V]])
                nc.sync.dma_start(out=dst_re, in_=flacc_re[rows, :, :])
                dst_im = bass.AP(tensor=fl_d.tensor,
                                 offset=S * NL * V + sl * NL * V,
                                 ap=[[V, NL], [SL * NL * V, SH], [1, V]])
                nc.sync.dma_start(out=dst_im, in_=flacc_im[rows, :, :])
    return nc


_CACHE = {}


def _get_program():
    if "nc" not in _CACHE:
        nc = bacc.Bacc("TRN2", target_bir_lowering=False, debug=False,
                       enable_asserts=False, num_devices=NCORES)
        _build(nc)
        nc.compile()   # DCE + wait-splitting (TRN2 allows 1 wait/instruction)
        _CACHE["nc"] = nc
    return _CACHE["nc"]


def _host_tables(eta, s, tau_stars):
    s128 = np.asarray(s, dtype=np.complex128)
    eta128 = np.asarray(eta, dtype=np.complex128)
    tau = np.asarray(tau_stars, dtype=np.float64)
    scale = (tau / NCONST) ** GEXP / tau
    K = -1.0 / s128
    etaK = eta128[None, :] * scale[:, None] * K      # folded output weights
    thr = SMALL_X / np.abs(s128)
    P0 = -s128
    P1 = s128 * s128 / 2.0
    P2 = -s128 ** 3 / 6.0
    sre = np.asarray(s).real
    assert np.abs(sre - sre[:, :1]).max() == 0.0, "s_re varies with v"

    def lay(arr_sv):
        out = np.empty((SH, 128, V), np.float32)
        for sh in range(SH):
            for sl in range(SL):
                out[sh, 32 * sl:32 * (sl + 1), :] = arr_sv[sh * SL + sl][None, :]
        return out

    tabs = {
        "sim": lay(s128.imag),
        "thr": lay(thr),
        "er": lay(etaK.real),
        "ei": lay(etaK.imag),
        "p0r": lay(P0.real), "p0i": lay(P0.imag),
        "p1r": lay(P1.real), "p1i": lay(P1.imag),
        "p2r": lay(P2.real), "p2i": lay(P2.imag),
        "kre": lay(K.real), "kim": lay(K.imag),
    }
    sren1 = np.empty((SH, 128), np.float32)
    for sh in range(SH):
        for sl in range(SL):
            sren1[sh, 32 * sl:32 * (sl + 1)] = -sre[sh * SL + sl, 0]
    tabs["sren1"] = sren1
    return tabs


def _shard_inputs(al2, fs2, c):
    cols = slice(NL * c, NL * (c + 1))
    return {"alpha_sh": np.ascontiguousarray(np.tile(al2[:, cols].T, (SL, 1))),
            "fs_sh": np.ascontiguousarray(np.tile(fs2[:, cols].T, (SL, 1)))}


def kernel(fs, alphas, eta, s, tau_stars):
    fs = np.asarray(fs, dtype=np.float32)
    alphas = np.asarray(alphas, dtype=np.float32)
    nc = _get_program()
    tabs = _host_tables(eta, s, tau_stars)

    al2 = np.ascontiguousarray(alphas.reshape(T, BF))
    fs2 = np.ascontiguousarray(fs.reshape(T, BF))
    in_maps = []
    for c in range(NCORES):
        m = _shard_inputs(al2, fs2, c)
        for n, arr in tabs.items():
            m[f"tab_{n}"] = arr
        in_maps.append(m)

    res = run_bass_kernel_spmd(nc, in_maps, core_ids=list(range(NCORES)))

    til = np.empty((T, BF, S), np.float32)
    fl = np.empty((BF, S, V), np.complex64)
    for c in range(NCORES):
        cols = slice(NL * c, NL * (c + 1))
        r = res.results[c]
        til[:, cols, :] = r["til_o"].transpose(2, 1, 0)   # [S,NL,T] -> [T,NL,S]
        flo = r["fl_o"]                                    # [2, S, NL, V]
        fl[cols] = (flo[0] + 1j * flo[1]).transpose(1, 0, 2)
    til_fs = til.reshape(T, B, F, S)
    f_last = fl.reshape(B, F, S, V)
    return til_fs, f_last
